# revision 4
# baseline (speedup 1.0000x reference)
"""Trainium2 Bass kernel for dynamic-filter conv routing (moe_routing).

Data parallel over batch: each of 8 cores handles 2 samples end-to-end.
Per sample: x1 = x + conv0(x); gx = GAP(x1); att = softmax(MLP(gx+gy));
agg kernel = att @ branch_kernels; out = dynconv(x1, agg) + x1 (same for y).

Conv3x3 as 18 accumulating PE matmuls per [128,512] output tile
(9 taps x 2 cin-chunks), inputs zero-padded to 66x66 on the host so every
tap is a uniform strided AP. Epilogue fuses bias + residual + GAP-partial
into one DVE scalar_tensor_tensor op.
"""
import sys
sys.path.insert(0, "/opt/trn_rl_repo")
import numpy as np

import concourse.bass as bass
import concourse.mybir as mybir
import concourse.tile as tile
from concourse.bass_utils import run_bass_kernel_spmd

F32 = mybir.dt.float32
N_CORES = 8
B, C, H, W = 16, 256, 64, 64
S = B // N_CORES          # samples per core
NB, KC = 4, 64
HP, WP = H + 2, W + 2     # padded spatial
PADN = HP * WP            # 4356
NT = 8                    # row-tiles per image (8 rows x 64 cols = 512)
AX = mybir.AxisListType
ALU = mybir.AluOpType
ACTF = mybir.ActivationFunctionType


def legalize_waits(nc):
    """This walrus build caps sync waits at 1/instruction (2 for
    EventSemaphore); hoist excess waits onto inserted EventSemaphores."""
    n_fix = 0
    for f in nc.m.functions:
        for bb in f.blocks:
            changed = False
            new_insts = []
            for inst in bb.instructions:
                si = inst.sync_info
                cap = 2 if isinstance(inst, mybir.InstEventSemaphore) else 1
                if si is not None and si.on_wait and len(si.on_wait) > cap:
                    waits = list(si.on_wait)
                    extra, keep = waits[:-cap], waits[-cap:]
                    for i in range(0, len(extra), 2):
                        ev = mybir.InstEventSemaphore(
                            name=nc.get_next_instruction_name(),
                            ins=[], outs=[], engine=inst.engine,
                            sync_info=mybir.SyncInfo(
                                on_wait=list(extra[i:i + 2]), on_update=[]),
                        )
                        new_insts.append(ev)
                    inst.sync_info = mybir.SyncInfo(
                        on_wait=list(keep), on_update=list(si.on_update or []))
                    changed = True
                    n_fix += 1
                new_insts.append(inst)
            if changed:
                bb.instructions = new_insts
    return n_fix


def build_nc(mm_dtype=mybir.dt.float32r):
    nc = bass.Bass()
    DT = mm_dtype

    # --- DRAM I/O (per core) ---
    xin = nc.declare_dram_parameter("xin", [S, C, PADN], DT, isOutput=False)
    yin = nc.declare_dram_parameter("yin", [S, C, PADN], DT, isOutput=False)
    wt = nc.declare_dram_parameter("wt", [2, 128, NB * 9 * 256], DT, isOutput=False)
    mvb = nc.declare_dram_parameter("mvb", [NB, 256], F32, isOutput=False)
    b0c = nc.declare_dram_parameter("b0c", [2, 128], F32, isOutput=False)
    wg = nc.declare_dram_parameter("wg", [2, 128, 512], F32, isOutput=False)
    b1 = nc.declare_dram_parameter("b1", [1, 512], F32, isOutput=False)
    q2t = nc.declare_dram_parameter("q2t", [4, 128, KC], F32, isOutput=False)
    mk20 = nc.declare_dram_parameter("mk20", [KC, NB], F32, isOutput=False)

    ox = nc.declare_dram_parameter("ox", [S, C, H * W], F32, isOutput=True)
    oy = nc.declare_dram_parameter("oy", [S, C, H * W], F32, isOutput=True)
    # padded intermediates (x1, y1); ExternalOutput => zero-donated buffers,
    # so the pad border is zero without any device-side memset.
    x1s = nc.declare_dram_parameter("x1s", [S, 2, 128, PADN], DT, isOutput=True)
    y1s = nc.declare_dram_parameter("y1s", [S, 2, 128, PADN], DT, isOutput=True)

    with tile.TileContext(nc) as tc:
        with (
            tc.tile_pool(name="wp", bufs=1) as wp,
            tc.tile_pool(name="pads", bufs=4) as pads,
            tc.tile_pool(name="aggp", bufs=3) as aggp,
            tc.tile_pool(name="outs", bufs=3) as outs,
            tc.tile_pool(name="smalls", bufs=1) as smalls,
            tc.tile_pool(name="gapp", bufs=4) as gapp,
            tc.tile_pool(name="mlpp", bufs=2) as mlpp,
            tc.tile_pool(name="cps", bufs=4, space="PSUM") as cps,
            tc.tile_pool(name="mps", bufs=3, space="PSUM") as mps,
        ):
            # ---- persistent weights / consts ----
            w_all = []
            for ch in range(2):
                wa = wp.tile([128, NB * 9 * 256], DT, name=f"w_all{ch}", tag=f"w{ch}")
                nc.sync.dma_start(out=wa[:, :], in_=wt[ch, :, :])
                w_all.append(wa)
            mvb_sb = smalls.tile([NB, 256], F32, name="mvb_sb", tag="mvb")
            nc.sync.dma_start(out=mvb_sb[:, :], in_=mvb[:, :])
            b0_sb = []
            for m_ch in range(2):
                t = smalls.tile([128, 1], F32, name=f"b0_{m_ch}", tag=f"b0_{m_ch}")
                nc.sync.dma_start(out=t[:, :], in_=b0c[m_ch, :])
                b0_sb.append(t)
            wg_sb = []
            for ch in range(2):
                t = smalls.tile([128, 512], F32, name=f"wg{ch}", tag=f"wg{ch}")
                nc.sync.dma_start(out=t[:, :], in_=wg[ch, :, :])
                wg_sb.append(t)
            b1_sb = smalls.tile([1, 512], F32, name="b1_sb", tag="b1")
            nc.sync.dma_start(out=b1_sb[:, :], in_=b1[:, :])
            q2_sb = []
            for j in range(4):
                t = smalls.tile([128, KC], F32, name=f"q2_{j}", tag=f"q2_{j}")
                nc.sync.dma_start(out=t[:, :], in_=q2t[j, :, :])
                q2_sb.append(t)
            mk_sb = smalls.tile([KC, NB], F32, name="mk_sb", tag="mk")
            nc.sync.dma_start(out=mk_sb[:, :], in_=mk20[:, :])
            ones_sb = smalls.tile([1, 512], F32, name="ones_sb", tag="ones")
            nc.vector.memset(ones_sb[:, :], 1.0)

            # ---- conv helper ----
            def conv(inpads, lhs_cols, bias_col, out_dma, gpart, use_f32r=True):
                """inpads: 2 chunk tiles [128,PADN] (padded input, also residual)
                lhs_cols(ch, t, m_ch) -> lhsT AP [128,128]
                bias_col: [128,1] AP per m_ch (callable)
                out_dma(m_ch, n, out_sb): emit store
                gpart: [128,16] tile for GAP partials or None
                """
                inr = [p.rearrange("p (r c) -> p r c", c=WP) for p in inpads]
                for m_ch in range(2):
                    for n in range(NT):
                        ps = cps.tile([128, 512], F32, name="cpsum", tag="cpsum")
                        first = True
                        for ch in range(2):
                            for t in range(9):
                                dy, dx = t // 3, t % 3
                                rhs = inr[ch][:, dy + n * 8: dy + n * 8 + 8, dx: dx + 64]
                                nc.tensor.matmul(
                                    ps[:, :], lhs_cols(ch, t, m_ch), rhs,
                                    start=first, stop=(ch == 1 and t == 8))
                                first = False
                        o = outs.tile([128, 512], F32, name="out_sb", tag="outs")
                        res = inr[m_ch][:, 1 + n * 8: 9 + n * 8, 1: 65]
                        nc.vector.scalar_tensor_tensor(
                            out=o[:, :], in0=ps[:, :], scalar=bias_col(m_ch),
                            in1=res, op0=ALU.add, op1=ALU.add,
                            accum_out=(gpart[:, m_ch * 8 + n: m_ch * 8 + n + 1]
                                       if gpart is not None else None))
                        out_dma(m_ch, n, o)

            def load_pads(src, s):
                ts = []
                for ch in range(2):
                    t = pads.tile([128, PADN], DT, name="pad", tag="pad")
                    nc.sync.dma_start(out=t[:, :], in_=src[s, ch * 128:(ch + 1) * 128, :])
                    ts.append(t)
                return ts

            def load_pads_scr(scr, s):
                ts = []
                for ch in range(2):
                    t = pads.tile([128, PADN], DT, name="pad", tag="pad")
                    nc.sync.dma_start(out=t[:, :], in_=scr[s, ch, :, :])
                    ts.append(t)
                return ts

            def w0_cols(ch, t, m_ch):
                return w_all[ch][:, t * 256 + m_ch * 128: t * 256 + m_ch * 128 + 128]

            def scr_dma(scr, s):
                def f(m_ch, n, o):
                    dst = scr[s, m_ch, :, :].rearrange("p (r c) -> p r c", c=WP)
                    nc.sync.dma_start(
                        out=dst[:, 1 + n * 8: 9 + n * 8, 1: 65],
                        in_=o[:, :].bitcast(DT))
                return f

            def out_dma(dest, s):
                def f(m_ch, n, o):
                    nc.sync.dma_start(
                        out=dest[s, m_ch * 128:(m_ch + 1) * 128,
                                 n * 512:(n + 1) * 512],
                        in_=o[:, :])
                return f

            # per-sample state
            gparts = {}
            aggs = {}
            aggb = {}

            def conv0_pair(s):
                xp = load_pads(xin, s)
                gx = gapp.tile([128, 16], F32, name="gx", tag="gap")
                conv(xp, w0_cols, lambda m: b0_sb[m][:, 0:1], scr_dma(x1s, s), gx)
                yp = load_pads(yin, s)
                gy = gapp.tile([128, 16], F32, name="gy", tag="gap")
                conv(yp, w0_cols, lambda m: b0_sb[m][:, 0:1], scr_dma(y1s, s), gy)
                gparts[s] = (gx, gy)

            def routing(s):
                gx, gy = gparts[s]
                g = mlpp.tile([128, 2], F32, name="g", tag="g")
                t1 = mlpp.tile([128, 2], F32, name="t1", tag="t1")
                for m_ch in range(2):
                    nc.vector.reduce_sum(
                        out=t1[:, m_ch:m_ch + 1],
                        in_=gx[:, m_ch * 8:(m_ch + 1) * 8], axis=AX.X)
                    nc.vector.reduce_sum(
                        out=g[:, m_ch:m_ch + 1],
                        in_=gy[:, m_ch * 8:(m_ch + 1) * 8], axis=AX.X)
                nc.vector.tensor_tensor(out=g[:, :], in0=g[:, :], in1=t1[:, :],
                                        op=ALU.add)
                # h = relu(g @ WgT + b1)   (1/HW folded into Wg host-side)
                hps = mps.tile([1, 512], F32, name="hps", tag="mps")
                for ch in range(2):
                    nc.tensor.matmul(hps[:, :], g[:, ch:ch + 1], wg_sb[ch][:, :],
                                     start=(ch == 0), stop=False)
                nc.tensor.matmul(hps[:, :], ones_sb[0:1, 0:1], b1_sb[:, :],
                                 start=False, stop=True)
                h = mlpp.tile([1, 512], F32, name="h", tag="h")
                nc.scalar.activation(h[:, :], hps[:, :], ACTF.Relu)
                # hT via PE transpose (4 x [1,128] -> [128,1])
                hTps = mps.tile([128, 4], F32, name="hTps", tag="mps")
                for j in range(4):
                    nc.tensor.matmul(hTps[:, j:j + 1],
                                     h[0:1, j * 128:(j + 1) * 128],
                                     ones_sb[0:1, 0:1],
                                     start=(j == 0), stop=(j == 3),
                                     is_transpose=True)
                hT = mlpp.tile([128, 4], F32, name="hT", tag="hT")
                nc.vector.tensor_copy(hT[:, :], hTps[:, :])
                # q = relu(h @ q2T)
                qps = mps.tile([1, KC], F32, name="qps", tag="mps")
                for j in range(4):
                    nc.tensor.matmul(qps[:, :], hT[:, j:j + 1], q2_sb[j][:, :],
                                     start=(j == 0), stop=(j == 3))
                q = mlpp.tile([1, KC], F32, name="q", tag="q")
                nc.scalar.activation(q[:, :], qps[:, :], ACTF.Relu)
                # qT
                qTps = mps.tile([KC, 1], F32, name="qTps", tag="mps")
                nc.tensor.matmul(qTps[:, :], q[0:1, :], ones_sb[0:1, 0:1],
                                 start=True, stop=True, is_transpose=True)
                qT = mlpp.tile([KC, 1], F32, name="qT", tag="qT")
                nc.vector.tensor_copy(qT[:, :], qTps[:, :])
                # logits = q @ mk/20 ; att = softmax(logits)
                lps = mps.tile([1, NB], F32, name="lps", tag="mps")
                nc.tensor.matmul(lps[:, :], qT[:, 0:1], mk_sb[:, :],
                                 start=True, stop=True)
                amax = mlpp.tile([1, 1], F32, name="amax", tag="amax")
                nc.vector.reduce_max(out=amax[:, :], in_=lps[:, :], axis=AX.X)
                aexp = mlpp.tile([1, NB], F32, name="aexp", tag="aexp")
                nc.vector.tensor_scalar_sub(aexp[:, :], lps[:, :], amax[:, 0:1])
                nc.scalar.activation(aexp[:, :], aexp[:, :], ACTF.Exp)
                asum = mlpp.tile([1, 1], F32, name="asum", tag="asum")
                nc.vector.reduce_sum(out=asum[:, :], in_=aexp[:, :], axis=AX.X)
                ainv = mlpp.tile([1, 1], F32, name="ainv", tag="ainv")
                nc.vector.reciprocal(ainv[:, :], asum[:, :])
                att = mlpp.tile([1, NB], F32, name="att", tag="att")
                nc.vector.tensor_scalar_mul(att[:, :], aexp[:, :], ainv[:, 0:1])
                # broadcast att across partitions: [128, NB]
                bcps = mps.tile([128, NB], F32, name="bcps", tag="mps")
                nc.tensor.matmul(bcps[:, :], ones_sb[0:1, 0:128], att[0:1, :],
                                 start=True, stop=True)
                attbc = mlpp.tile([128, NB], F32, name="attbc", tag="attbc")
                nc.vector.tensor_copy(attbc[:, :], bcps[:, :])
                # attT [NB,1]
                atps = mps.tile([NB, 1], F32, name="atps", tag="mps")
                nc.tensor.matmul(atps[:, :], att[0:1, :], ones_sb[0:1, 0:1],
                                 start=True, stop=True, is_transpose=True)
                attT = mlpp.tile([NB, 1], F32, name="attT", tag="attT")
                nc.vector.tensor_copy(attT[:, :], atps[:, :])
                # agg bias per cout chunk
                ab = []
                for m_ch in range(2):
                    abps = mps.tile([128, 1], F32, name="abps", tag="mps")
                    nc.tensor.matmul(abps[:, :],
                                     mvb_sb[:, m_ch * 128:(m_ch + 1) * 128],
                                     attT[:, 0:1], start=True, stop=True)
                    t = mlpp.tile([128, 1], F32, name=f"aggb{m_ch}",
                                  tag=f"aggb{m_ch}")
                    nc.vector.tensor_copy(t[:, :], abps[:, :])
                    ab.append(t)
                aggb[s] = ab
                # aggregated conv kernel, wT layout [128, 9*256] per chunk
                ag = []
                for ch in range(2):
                    a = aggp.tile([128, 9 * 256], DT, name="agg", tag="agg")
                    nc.vector.tensor_scalar_mul(
                        a[:, :], w_all[ch][:, 0:2304], attbc[:, 0:1])
                    for nb in range(1, NB):
                        nc.vector.scalar_tensor_tensor(
                            out=a[:, :],
                            in0=w_all[ch][:, nb * 2304:(nb + 1) * 2304],
                            scalar=attbc[:, nb:nb + 1], in1=a[:, :],
                            op0=ALU.mult, op1=ALU.add)
                    ag.append(a)
                aggs[s] = ag

            def dyn_pair(s):
                ag = aggs[s]
                ab = aggb[s]

                def agg_cols(ch, t, m_ch):
                    return ag[ch][:, t * 256 + m_ch * 128: t * 256 + m_ch * 128 + 128]

                xp = load_pads_scr(x1s, s)
                conv(xp, agg_cols, lambda m: ab[m][:, 0:1], out_dma(ox, s), None)
                yp = load_pads_scr(y1s, s)
                conv(yp, agg_cols, lambda m: ab[m][:, 0:1], out_dma(oy, s), None)

            # ---- schedule: conv0 all samples, routing interleaved, dyn ----
            conv0_pair(0)
            routing(0)
            conv0_pair(1)
            dyn_pair(0)
            routing(1)
            dyn_pair(1)

    legalize_waits(nc)
    return nc


def host_prep(x, y, scale, mv_w, mv_b, q1_w, q1_b, q2_w, key_w, flag):
    """Host-side input prep: pad/transpose/fold. Returns per-core in_maps."""
    if not flag:
        # reference's flag=0 path is shape-invalid for B!=2; only flag!=0 valid
        raise ValueError("flag==0 unsupported (reference is shape-invalid)")
    scale_info = np.broadcast_to(scale.reshape(1, 2), (B, 2))[0]  # same per row

    xp = np.zeros((B, C, HP, WP), np.float32)
    xp[:, :, 1:65, 1:65] = x
    yp = np.zeros((B, C, HP, WP), np.float32)
    yp[:, :, 1:65, 1:65] = y
    xp = xp.reshape(B, C, PADN)
    yp = yp.reshape(B, C, PADN)

    # wt[ch, p, nb*2304 + t*256 + co] = mv_w[nb, co, ch*128+p, t//3, t%3]
    wtr = mv_w.reshape(NB, 256, 256, 9).transpose(2, 0, 3, 1)  # cin, nb, t, co
    wt = np.ascontiguousarray(wtr.reshape(2, 128, NB * 9 * 256), np.float32)

    b0cv = np.ascontiguousarray(mv_b[0].reshape(2, 128), np.float32)
    # Wg: q1_w[:, :C] scaled by 1/(H*W); b1_eff = q1_b + scale_info @ Ws^T
    q1 = q1_w.reshape(2 * C, C + 2)
    wgv = np.ascontiguousarray(
        (q1[:, :C].T / float(H * W)).reshape(2, 128, 512), np.float32)
    b1v = (q1_b + scale_info @ q1[:, C:].T).reshape(1, 512).astype(np.float32)
    q2tv = np.ascontiguousarray(
        q2_w.reshape(KC, 2 * C).T.reshape(4, 128, KC), np.float32)
    mk20v = (key_w.reshape(KC, NB) / 20.0).astype(np.float32)
    mvbv = np.ascontiguousarray(mv_b, np.float32)

    in_maps = []
    for c in range(N_CORES):
        sl = slice(c * S, (c + 1) * S)
        in_maps.append({
            "xin": np.ascontiguousarray(xp[sl]),
            "yin": np.ascontiguousarray(yp[sl]),
            "wt": wt, "mvb": mvbv, "b0c": b0cv, "wg": wgv, "b1": b1v,
            "q2t": q2tv, "mk20": mk20v,
        })
    return in_maps


_NC_CACHE = {}


def get_nc(mm_dtype=mybir.dt.float32r):
    key = str(mm_dtype)
    if key not in _NC_CACHE:
        _NC_CACHE[key] = build_nc(mm_dtype)
    return _NC_CACHE[key]


def kernel(x, y, scale, mv_w, mv_b, q1_w, q1_b, q2_w, q2_b, key_w, flag):
    x = np.asarray(x, np.float32)
    y = np.asarray(y, np.float32)
    scale = np.asarray(scale, np.float32)
    mv_w = np.asarray(mv_w, np.float32)
    mv_b = np.asarray(mv_b, np.float32)
    q1_w = np.asarray(q1_w, np.float32)
    q1_b = np.asarray(q1_b, np.float32)
    q2_w = np.asarray(q2_w, np.float32)
    key_w = np.asarray(key_w, np.float32)
    flag = int(np.asarray(flag))

    in_maps = host_prep(x, y, scale, mv_w, mv_b, q1_w, q1_b, q2_w, key_w, flag)
    nc = get_nc()
    res = run_bass_kernel_spmd(nc, in_maps, list(range(N_CORES)))

    xo = np.empty((B, C, H, W), np.float32)
    yo = np.empty((B, C, H, W), np.float32)
    for c in range(N_CORES):
        xo[c * S:(c + 1) * S] = res.results[c]["ox"].reshape(S, C, H, W)
        yo[c * S:(c + 1) * S] = res.results[c]["oy"].reshape(S, C, H, W)
    return (xo, yo)


# revision 14
# speedup vs baseline: 27308.6177x; 27308.6177x over previous
"""Trainium2 Bass kernel for dynamic-filter conv routing (moe_routing).

Data parallel over batch: each of 8 cores handles 2 samples end-to-end.
Per sample: x1 = x + conv0(x); gx = GAP(x1); att = softmax(MLP(gx+gy));
agg kernel = att @ branch_kernels; out = dynconv(x1, agg) + x1 (same for y).

Conv3x3 as 18 accumulating PE matmuls per [128,512] output tile
(9 taps x 2 cin-chunks), inputs zero-padded to 66x66 on the host so every
tap is a uniform strided AP. Epilogue fuses bias + residual + GAP-partial
into one DVE scalar_tensor_tensor op.
"""
import sys
sys.path.insert(0, "/opt/trn_rl_repo")
import numpy as np

import concourse.bass as bass
import concourse.mybir as mybir
import concourse.tile as tile
from concourse.bass_utils import run_bass_kernel_spmd

F32 = mybir.dt.float32
N_CORES = 8
B, C, H, W = 16, 256, 64, 64
S = B // N_CORES          # samples per core
NB, KC = 4, 64
HP, WP = H + 2, W + 2     # padded spatial
PADN = HP * WP            # 4356
NT = 8                    # row-tiles per image (8 rows x 64 cols = 512)
AX = mybir.AxisListType
ALU = mybir.AluOpType
ACTF = mybir.ActivationFunctionType


def legalize_waits(nc):
    """This walrus build caps sync waits at 1/instruction (2 for
    EventSemaphore); hoist excess waits onto inserted EventSemaphores."""
    n_fix = 0
    for f in nc.m.functions:
        for bb in f.blocks:
            changed = False
            new_insts = []
            for inst in bb.instructions:
                si = inst.sync_info
                cap = 2 if isinstance(inst, mybir.InstEventSemaphore) else 1
                if si is not None and si.on_wait and len(si.on_wait) > cap:
                    waits = list(si.on_wait)
                    extra, keep = waits[:-cap], waits[-cap:]
                    for i in range(0, len(extra), 2):
                        ev = mybir.InstEventSemaphore(
                            name=nc.get_next_instruction_name(),
                            ins=[], outs=[], engine=inst.engine,
                            sync_info=mybir.SyncInfo(
                                on_wait=list(extra[i:i + 2]), on_update=[]),
                        )
                        new_insts.append(ev)
                    inst.sync_info = mybir.SyncInfo(
                        on_wait=list(keep), on_update=list(si.on_update or []))
                    changed = True
                    n_fix += 1
                new_insts.append(inst)
            if changed:
                bb.instructions = new_insts
    return n_fix


def build_nc(mm_dtype=mybir.dt.float32r, loop_reps=1, external_io=True, skeleton=False):
    nc = bass.Bass()
    DT = mm_dtype

    # --- DRAM I/O (per core) ---
    if external_io:
        dram_in = lambda nm, sh, dt: nc.declare_dram_parameter(nm, sh, dt, isOutput=False)
        dram_out = lambda nm, sh, dt: nc.declare_dram_parameter(nm, sh, dt, isOutput=True)
    else:
        dram_in = lambda nm, sh, dt: nc.dram_tensor(nm, sh, dt)
        dram_out = lambda nm, sh, dt: nc.dram_tensor(nm, sh, dt)
        nc.declare_dram_parameter("din", [1, 1 + loop_reps], F32, isOutput=False)
        dout = nc.declare_dram_parameter("dout", [1, 1], F32, isOutput=True)
    xin = dram_in("xin", [S, C, PADN], DT)
    yin = dram_in("yin", [S, C, PADN], DT)
    wt = dram_in("wt", [2, 128, NB * 9 * 256], DT)
    mvb = dram_in("mvb", [NB, 256], F32)
    b0c = dram_in("b0c", [2, 128], F32)
    wg = dram_in("wg", [2, 128, 512], F32)
    b1 = dram_in("b1", [1, 512], F32)
    q2t = dram_in("q2t", [4, 128, KC], F32)
    mk20 = dram_in("mk20", [KC, NB], F32)

    ox = dram_out("ox", [S, C, H * W], F32)
    oy = dram_out("oy", [S, C, H * W], F32)
    # padded intermediates (x1, y1); ExternalOutput => zero-donated buffers,
    # so the pad border is zero without any device-side memset.
    x1s = dram_out("x1s", [S, 2, 128, 64 * WP], DT)
    y1s = dram_out("y1s", [S, 2, 128, 64 * WP], DT)

    with tile.TileContext(nc) as tc:
        with (
            tc.tile_pool(name="wp", bufs=1) as wp,
            tc.tile_pool(name="pads", bufs=1) as pads,
            tc.tile_pool(name="aggp", bufs=4) as aggp,
            tc.tile_pool(name="outs", bufs=2) as outs,
            tc.tile_pool(name="smalls", bufs=1) as smalls,
            tc.tile_pool(name="gapp", bufs=4) as gapp,
            tc.tile_pool(name="mlpp", bufs=1) as mlpp,
            tc.tile_pool(name="cps", bufs=6, space="PSUM") as cps,
            tc.tile_pool(name="mps", bufs=2, space="PSUM") as mps,
        ):
            # ---- persistent weights / consts ----
            def body():
              w_all = []
              for ch in range(2):
                  wa = wp.tile([128, NB * 9 * 256], DT, name=f"w_all{ch}", tag=f"w{ch}")
                  w_all.append(wa)

              def load_w(nb, chs=(0, 1)):
                  for ch in chs:
                      nc.sync.dma_start(
                          out=w_all[ch][:, nb * 2304:(nb + 1) * 2304],
                          in_=wt[ch, :, nb * 2304:(nb + 1) * 2304])
              b0_sb = [smalls.tile([128, 1], F32, name=f"b0_{m_ch}",
                                   tag=f"b0_{m_ch}") for m_ch in range(2)]

              def load_b0():
                  for m_ch in range(2):
                      nc.sync.dma_start(out=b0_sb[m_ch][:, :], in_=b0c[m_ch, :])
              mvb_sb = smalls.tile([NB, 256], F32, name="mvb_sb", tag="mvb")
              wg_sb = [smalls.tile([128, 512], F32, name=f"wg{ch}", tag=f"wg{ch}")
                       for ch in range(2)]
              b1_sb = smalls.tile([1, 512], F32, name="b1_sb", tag="b1")
              q2_sb = [smalls.tile([128, KC], F32, name=f"q2_{j}", tag=f"q2_{j}")
                       for j in range(4)]
              mk_sb = smalls.tile([KC, NB], F32, name="mk_sb", tag="mk")
              ones_sb = smalls.tile([1, 128], F32, name="ones_sb", tag="ones")

              def load_smalls():
                  nc.sync.dma_start(out=mvb_sb[:, :], in_=mvb[:, :])
                  for ch in range(2):
                      nc.sync.dma_start(out=wg_sb[ch][:, :], in_=wg[ch, :, :])
                  nc.sync.dma_start(out=b1_sb[:, :], in_=b1[:, :])
                  for j in range(4):
                      nc.sync.dma_start(out=q2_sb[j][:, :], in_=q2t[j, :, :])
                  nc.sync.dma_start(out=mk_sb[:, :], in_=mk20[:, :])
                  nc.vector.memset(ones_sb[:, :], 1.0)

              # ---- conv helper ----
              # persistent 66-wide out tiles with pre-zeroed border columns
              po_tiles = [outs.tile([128, 16 * WP], F32, name=f"po{i}",
                                    tag=f"po{i}", bufs=1) for i in range(2)]
              for i in range(2):
                  nc.vector.memset(po_tiles[i][:, :], 0.0)
              po_rot = [0]

              def conv(inpads, lhs_cols, bias_col, out_dma, gpart,
                       pad_out=False):
                  """inpads: 2 chunk tiles [128,PADN] (padded input+residual)
                  lhs_cols(ch, t, m_ch) -> lhsT AP [128,128]
                  bias_col: [128,1] AP per m_ch (callable)
                  out_dma(m_ch, n, out_sb): emit store
                  gpart: [128,16] tile for GAP partials or None
                  pad_out: write 66-wide rows (zero border cols) so the
                  store is one contiguous DMA
                  """
                  inr = [p.rearrange("p (r c) -> p r c", c=WP) for p in inpads]
                  o_prev = None
                  for m_ch in range(2):
                      for n in range(NT):
                          ps = cps.tile([128, 512], F32, name="cpsum", tag="cpsum")
                          first = True
                          for ch in range(2):
                              for t in range(9):
                                  dy, dx = t // 3, t % 3
                                  rhs = inr[ch][:, dy + n * 8: dy + n * 8 + 8, dx: dx + 64]
                                  nc.tensor.matmul(
                                      ps[:, :], lhs_cols(ch, t, m_ch), rhs,
                                      start=first, stop=(ch == 1 and t == 8))
                                  first = False
                          if skeleton:
                              continue
                          half = n % 2
                          if pad_out:
                              if half == 0:
                                  o = po_tiles[po_rot[0] % 2]
                                  po_rot[0] += 1
                              else:
                                  o = po_tiles[(po_rot[0] - 1) % 2]
                              ow = o.rearrange("p (r c) -> p r c", c=WP)[
                                  :, half * 8:(half + 1) * 8, 1:65]
                          else:
                              if half == 0:
                                  o = outs.tile([128, 1024], F32,
                                                name="out_sb", tag="outs")
                                  o_prev = o
                              else:
                                  o = o_prev
                              ow = o[:, half * 512:(half + 1) * 512]
                          res = inr[m_ch][:, 1 + n * 8: 9 + n * 8, 1: 65]
                          nc.vector.scalar_tensor_tensor(
                              out=ow, in0=ps[:, :], scalar=bias_col(m_ch),
                              in1=res, op0=ALU.add, op1=ALU.add,
                              accum_out=(gpart[:, m_ch * 8 + n: m_ch * 8 + n + 1]
                                         if gpart is not None else None))
                          if half == 1:
                              out_dma(m_ch, n - 1, o)

              BANDS = [(0, 17), (17, 34), (34, 50), (50, 66)]
              SBANDS = [(0, 16), (16, 32), (32, 48), (48, 64)]

              pad_tiles = [pads.tile([128, PADN], DT, name=f"padt{i}",
                                     tag=f"padt{i}") for i in range(4)]
              for i in range(4):
                  # zero top/bottom pad rows once; every later write either
                  # covers them (full input load) or leaves them (scratch)
                  nc.vector.memset(pad_tiles[i][:, 0:WP].bitcast(F32), 0.0)
                  nc.vector.memset(
                      pad_tiles[i][:, 65 * WP:66 * WP].bitcast(F32), 0.0)
              pad_rot = [0]

              def next_pads():
                  ts = [pad_tiles[pad_rot[0] % 4], pad_tiles[(pad_rot[0] + 1) % 4]]
                  pad_rot[0] += 2
                  return ts

              def load_pads(src, s):
                  ts = next_pads()
                  for r0, r1 in BANDS:
                      for ch in range(2):
                          nc.sync.dma_start(
                              out=ts[ch][:, r0 * WP:r1 * WP],
                              in_=src[s, ch * 128:(ch + 1) * 128, r0 * WP:r1 * WP])
                  return ts

              def load_pads_scr(scr, s):
                  ts = next_pads()
                  for r0, r1 in SBANDS:
                      for ch in range(2):
                          nc.sync.dma_start(
                              out=ts[ch][:, (r0 + 1) * WP:(r1 + 1) * WP],
                              in_=scr[s, ch, :, r0 * WP:r1 * WP])
                  return ts

              def w0_cols(ch, t, m_ch):
                  return w_all[ch][:, t * 256 + m_ch * 128: t * 256 + m_ch * 128 + 128]

              def scr_dma(scr, s):
                  def f(m_ch, n, o):
                      nc.sync.dma_start(
                          out=scr[s, m_ch, :, n * 8 * WP:(n + 2) * 8 * WP],
                          in_=o[:, :].bitcast(DT))
                  return f

              def out_dma(dest, s):
                  def f(m_ch, n, o):
                      nc.sync.dma_start(
                          out=dest[s, m_ch * 128:(m_ch + 1) * 128,
                                   n * 512:(n + 2) * 512],
                          in_=o[:, :])
                  return f

              # per-sample state
              gparts = {}
              aggs = {}
              aggb = {}

              def conv0_one(src_t, scr_t, s, gtile, preloaded=None):
                  p = preloaded if preloaded is not None else load_pads(src_t, s)
                  conv(p, w0_cols, lambda m: b0_sb[m][:, 0:1],
                       scr_dma(scr_t, s), gtile, pad_out=True)

              def conv0_pair(s):
                  gx = gapp.tile([128, 16], F32, name="gx", tag="gap")
                  conv0_one(xin, x1s, s, gx)
                  gy = gapp.tile([128, 16], F32, name="gy", tag="gap")
                  conv0_one(yin, y1s, s, gy)
                  gparts[s] = (gx, gy)

              def routing(s):
                  gx, gy = gparts[s]
                  g = mlpp.tile([128, 2], F32, name="g", tag="g")
                  t1 = mlpp.tile([128, 2], F32, name="t1", tag="t1")
                  for m_ch in range(2):
                      nc.vector.reduce_sum(
                          out=t1[:, m_ch:m_ch + 1],
                          in_=gx[:, m_ch * 8:(m_ch + 1) * 8], axis=AX.X)
                      nc.vector.reduce_sum(
                          out=g[:, m_ch:m_ch + 1],
                          in_=gy[:, m_ch * 8:(m_ch + 1) * 8], axis=AX.X)
                  nc.vector.tensor_tensor(out=g[:, :], in0=g[:, :], in1=t1[:, :],
                                          op=ALU.add)
                  # h = relu(g @ WgT + b1)   (1/HW folded into Wg host-side)
                  hps = mps.tile([1, 512], F32, name="hps", tag="mps")
                  for ch in range(2):
                      nc.tensor.matmul(hps[:, :], g[:, ch:ch + 1], wg_sb[ch][:, :],
                                       start=(ch == 0), stop=False)
                  nc.tensor.matmul(hps[:, :], ones_sb[0:1, 0:1], b1_sb[:, :],
                                   start=False, stop=True)
                  h = mlpp.tile([1, 512], F32, name="h", tag="h")
                  nc.scalar.activation(h[:, :], hps[:, :], ACTF.Relu)
                  # hT via PE transpose (4 x [1,128] -> [128,1])
                  hTps = mps.tile([128, 4], F32, name="hTps", tag="mps")
                  for j in range(4):
                      nc.tensor.matmul(hTps[:, j:j + 1],
                                       h[0:1, j * 128:(j + 1) * 128],
                                       ones_sb[0:1, 0:1],
                                       start=(j == 0), stop=(j == 3),
                                       is_transpose=True)
                  hT = mlpp.tile([128, 4], F32, name="hT", tag="hT")
                  nc.vector.tensor_copy(hT[:, :], hTps[:, :])
                  # q = relu(h @ q2T)
                  qps = mps.tile([1, KC], F32, name="qps", tag="mps")
                  for j in range(4):
                      nc.tensor.matmul(qps[:, :], hT[:, j:j + 1], q2_sb[j][:, :],
                                       start=(j == 0), stop=(j == 3))
                  q = mlpp.tile([1, KC], F32, name="q", tag="q")
                  nc.scalar.activation(q[:, :], qps[:, :], ACTF.Relu)
                  # qT
                  qTps = mps.tile([KC, 1], F32, name="qTps", tag="mps")
                  nc.tensor.matmul(qTps[:, :], q[0:1, :], ones_sb[0:1, 0:1],
                                   start=True, stop=True, is_transpose=True)
                  qT = mlpp.tile([KC, 1], F32, name="qT", tag="qT")
                  nc.vector.tensor_copy(qT[:, :], qTps[:, :])
                  # logits = q @ mk/20 ; att = softmax(logits)
                  lps = mps.tile([1, NB], F32, name="lps", tag="mps")
                  nc.tensor.matmul(lps[:, :], qT[:, 0:1], mk_sb[:, :],
                                   start=True, stop=True)
                  amax = mlpp.tile([1, 1], F32, name="amax", tag="amax")
                  nc.vector.reduce_max(out=amax[:, :], in_=lps[:, :], axis=AX.X)
                  aexp = mlpp.tile([1, NB], F32, name="aexp", tag="aexp")
                  nc.vector.tensor_scalar_sub(aexp[:, :], lps[:, :], amax[:, 0:1])
                  nc.scalar.activation(aexp[:, :], aexp[:, :], ACTF.Exp)
                  asum = mlpp.tile([1, 1], F32, name="asum", tag="asum")
                  nc.vector.reduce_sum(out=asum[:, :], in_=aexp[:, :], axis=AX.X)
                  ainv = mlpp.tile([1, 1], F32, name="ainv", tag="ainv")
                  nc.vector.reciprocal(ainv[:, :], asum[:, :])
                  att = mlpp.tile([1, NB], F32, name="att", tag="att")
                  nc.vector.tensor_scalar_mul(att[:, :], aexp[:, :], ainv[:, 0:1])
                  # broadcast att across partitions: [128, NB]
                  bcps = mps.tile([128, NB], F32, name="bcps", tag="mps")
                  nc.tensor.matmul(bcps[:, :], ones_sb[0:1, 0:128], att[0:1, :],
                                   start=True, stop=True)
                  attbc = mlpp.tile([128, NB], F32, name="attbc", tag="attbc")
                  nc.vector.tensor_copy(attbc[:, :], bcps[:, :])
                  # attT [NB,1]
                  atps = mps.tile([NB, 1], F32, name="atps", tag="mps")
                  nc.tensor.matmul(atps[:, :], att[0:1, :], ones_sb[0:1, 0:1],
                                   start=True, stop=True, is_transpose=True)
                  attT = mlpp.tile([NB, 1], F32, name="attT", tag="attT")
                  nc.vector.tensor_copy(attT[:, :], atps[:, :])
                  # agg bias per cout chunk
                  ab = []
                  for m_ch in range(2):
                      abps = mps.tile([128, 1], F32, name="abps", tag="mps")
                      nc.tensor.matmul(abps[:, :],
                                       mvb_sb[:, m_ch * 128:(m_ch + 1) * 128],
                                       attT[:, 0:1], start=True, stop=True)
                      t = mlpp.tile([128, 1], F32, name=f"aggb{m_ch}",
                                    tag=f"aggb{m_ch}", bufs=2)
                      nc.vector.tensor_copy(t[:, :], abps[:, :])
                      ab.append(t)
                  aggb[s] = ab
                  # aggregated conv kernel, wT layout [128, 9*256] per chunk
                  ag = []
                  for ch in range(2):
                      a = aggp.tile([128, 9 * 256], DT, name="agg", tag="agg")
                      nc.vector.tensor_scalar_mul(
                          a[:, :], w_all[ch][:, 0:2304], attbc[:, 0:1])
                      for nb in range(1, NB):
                          nc.vector.scalar_tensor_tensor(
                              out=a[:, :],
                              in0=w_all[ch][:, nb * 2304:(nb + 1) * 2304],
                              scalar=attbc[:, nb:nb + 1], in1=a[:, :],
                              op0=ALU.mult, op1=ALU.add)
                      ag.append(a)
                  aggs[s] = ag

              def dyn_pair(s):
                  ag = aggs[s]
                  ab = aggb[s]

                  def agg_cols(ch, t, m_ch):
                      return ag[ch][:, t * 256 + m_ch * 128: t * 256 + m_ch * 128 + 128]

                  xp = load_pads_scr(x1s, s)
                  conv(xp, agg_cols, lambda m: ab[m][:, 0:1], out_dma(ox, s), None)
                  yp = load_pads_scr(y1s, s)
                  conv(yp, agg_cols, lambda m: ab[m][:, 0:1], out_dma(oy, s), None)

              # schedule: first weight half + first input band lead the DMA
              # queue so PE starts ~4us in; everything else rides behind.
              w0r = [w_all[ch][:, 0:2304].rearrange("p (t m) -> p t m", m=256)
                     for ch in range(2)]
              wt0r = [wt[ch, :, 0:2304].rearrange("p (t m) -> p t m", m=256)
                      for ch in range(2)]
              nc.sync.dma_start(out=w0r[0][:, :, 0:128], in_=wt0r[0][:, :, 0:128])
              xp0 = next_pads()
              for bi, (r0, r1) in enumerate(BANDS):
                  for ch in range(2):
                      nc.sync.dma_start(
                          out=xp0[ch][:, r0 * WP:r1 * WP],
                          in_=xin[0, ch * 128:(ch + 1) * 128, r0 * WP:r1 * WP])
                  if bi == 0:
                      nc.sync.dma_start(out=w0r[1][:, :, 0:128],
                                        in_=wt0r[1][:, :, 0:128])
                      load_b0()
                  elif bi == 1:
                      nc.sync.dma_start(out=w0r[0][:, :, 128:256],
                                        in_=wt0r[0][:, :, 128:256])
                      nc.sync.dma_start(out=w0r[1][:, :, 128:256],
                                        in_=wt0r[1][:, :, 128:256])
              yp0 = load_pads(yin, 0)
              gx0 = gapp.tile([128, 16], F32, name="gx", tag="gap")
              conv0_one(xin, x1s, 0, gx0, preloaded=xp0)
              for nb in range(1, NB):
                  load_w(nb)
              load_smalls()
              gy0 = gapp.tile([128, 16], F32, name="gy", tag="gap")
              conv0_one(yin, y1s, 0, gy0, preloaded=yp0)
              gparts[0] = (gx0, gy0)
              if not skeleton:
                  routing(0)
              else:
                  aggs[0] = w_all
                  aggb[0] = b0_sb
              conv0_pair(1)
              if not skeleton:
                  routing(1)
              else:
                  aggs[0] = w_all
                  aggs[1] = w_all
                  aggb[0] = b0_sb
                  aggb[1] = b0_sb
              dyn_pair(0)
              dyn_pair(1)
              if not external_io:
                  nc.sync.dma_start(out=dout[:, :], in_=ones_sb[0:1, 0:1])

            if loop_reps > 1:
                with tc.For_i(0, loop_reps, 1,
                              hint_engines=(mybir.EngineType.PE,
                                            mybir.EngineType.SP,
                                            mybir.EngineType.DVE)):
                    body()
            else:
                body()

    legalize_waits(nc)
    return nc


def host_prep(x, y, scale, mv_w, mv_b, q1_w, q1_b, q2_w, key_w, flag):
    """Host-side input prep: pad/transpose/fold. Returns per-core in_maps."""
    if not flag:
        # reference's flag=0 path is shape-invalid for B!=2; only flag!=0 valid
        raise ValueError("flag==0 unsupported (reference is shape-invalid)")
    scale_info = np.broadcast_to(scale.reshape(1, 2), (B, 2))[0]  # same per row

    xp = np.zeros((B, C, HP, WP), np.float32)
    xp[:, :, 1:65, 1:65] = x
    yp = np.zeros((B, C, HP, WP), np.float32)
    yp[:, :, 1:65, 1:65] = y
    xp = xp.reshape(B, C, PADN)
    yp = yp.reshape(B, C, PADN)

    # wt[ch, p, nb*2304 + t*256 + co] = mv_w[nb, co, ch*128+p, t//3, t%3]
    wtr = mv_w.reshape(NB, 256, 256, 9).transpose(2, 0, 3, 1)  # cin, nb, t, co
    wt = np.ascontiguousarray(wtr.reshape(2, 128, NB * 9 * 256), np.float32)

    b0cv = np.ascontiguousarray(mv_b[0].reshape(2, 128), np.float32)
    # Wg: q1_w[:, :C] scaled by 1/(H*W); b1_eff = q1_b + scale_info @ Ws^T
    q1 = q1_w.reshape(2 * C, C + 2)
    wgv = np.ascontiguousarray(
        (q1[:, :C].T / float(H * W)).reshape(2, 128, 512), np.float32)
    b1v = (q1_b + scale_info @ q1[:, C:].T).reshape(1, 512).astype(np.float32)
    q2tv = np.ascontiguousarray(
        q2_w.reshape(KC, 2 * C).T.reshape(4, 128, KC), np.float32)
    mk20v = (key_w.reshape(KC, NB) / 20.0).astype(np.float32)
    mvbv = np.ascontiguousarray(mv_b, np.float32)

    in_maps = []
    for c in range(N_CORES):
        sl = slice(c * S, (c + 1) * S)
        in_maps.append({
            "xin": np.ascontiguousarray(xp[sl]),
            "yin": np.ascontiguousarray(yp[sl]),
            "wt": wt, "mvb": mvbv, "b0c": b0cv, "wg": wgv, "b1": b1v,
            "q2t": q2tv, "mk20": mk20v,
        })
    return in_maps


_NC_CACHE = {}


def get_nc(mm_dtype=mybir.dt.float32r):
    key = str(mm_dtype)
    if key not in _NC_CACHE:
        _NC_CACHE[key] = build_nc(mm_dtype)
    return _NC_CACHE[key]


def kernel(x, y, scale, mv_w, mv_b, q1_w, q1_b, q2_w, q2_b, key_w, flag):
    x = np.asarray(x, np.float32)
    y = np.asarray(y, np.float32)
    scale = np.asarray(scale, np.float32)
    mv_w = np.asarray(mv_w, np.float32)
    mv_b = np.asarray(mv_b, np.float32)
    q1_w = np.asarray(q1_w, np.float32)
    q1_b = np.asarray(q1_b, np.float32)
    q2_w = np.asarray(q2_w, np.float32)
    key_w = np.asarray(key_w, np.float32)
    flag = int(np.asarray(flag))

    in_maps = host_prep(x, y, scale, mv_w, mv_b, q1_w, q1_b, q2_w, key_w, flag)
    nc = get_nc()
    res = run_bass_kernel_spmd(nc, in_maps, list(range(N_CORES)))

    xo = np.empty((B, C, H, W), np.float32)
    yo = np.empty((B, C, H, W), np.float32)
    for c in range(N_CORES):
        xo[c * S:(c + 1) * S] = res.results[c]["ox"].reshape(S, C, H, W)
        yo[c * S:(c + 1) * S] = res.results[c]["oy"].reshape(S, C, H, W)
    return (xo, yo)

